# revision 1
# baseline (speedup 1.0000x reference)
"""Trainium2 Bass kernel for nn_MultiHeadAttention (softmax over HEAD axis).

Problem: B=2, T=2048, D=1024, H=16, HD=64.
  Q,K,V = x@W* + b*;  score = QK^T/32 with causal positions set to -1e10
  weight = softmax(score, axis=HEADS)  -> masked (j>i) entries get exactly 1/16
  out = weight@V;  y = out@Wo + bo

Exact identity used: for row i,
  out_h[i] = sum_{j<=i} w_h[i,j] V_h[j] + (1/16) sum_{j>i} V_h[j]
where w is the head-softmax of unmasked scores. We compute softmax weights
only on causal j-blocks, zero the off-causal entries via 0/1 masks, and add
the (1/16)*suffix-sum(V) correction as a host-precomputed matrix (V comes
from launch A's own output, so the correction is consistent to fp16).

Sharding (8 cores, two launches):
  Launch A: QKV projections, 8-way token-sharded.
  Launch B: attention + out-proj. Core c (q = c%4, batch c//4) handles the
    mirrored 2-block chunks A=(2q, 2q+1), B=(14-2q, 15-2q) of 128-row blocks.
    One SPMD program for all cores: slot A runs 8 j-block positions, slot B
    16; real causal counts are (2q+2, 16-2q) and the rest are padding whose
    weights the per-core mask data zeroes. Total exact work is equal on all
    cores (18 positions); padding adds 6.

All matmul inputs fp16 (1 cyc/row on PE), accumulation fp32 in PSUM.
"""

import numpy as np

import concourse.bass as bass
import concourse.tile as tile
from concourse import bacc, mybir
from concourse.bass_utils import run_bass_kernel_spmd

F16 = mybir.dt.float16
F32 = mybir.dt.float32
AF = mybir.ActivationFunctionType

B, T, D, H, HD = 2, 2048, 1024, 16, 64
NC = 8
NBLK = T // 128          # 16
CNT = (8, 16)            # padded j-position counts for slot A / slot B
NPOS = CNT[0] + CNT[1]   # 24

# head slot order per 4-head score group: even (row-group-0) heads first so
# each PSUM bank only ever receives matmuls from one PE row group.
_GRP_HEADS = [[4 * g, 4 * g + 2, 4 * g + 1, 4 * g + 3] for g in range(4)]
# head -> (group, slot)
_HEAD_SLOT = {}
for _g in range(4):
    for _s, _h in enumerate(_GRP_HEADS[_g]):
        _HEAD_SLOT[_h] = (_g, _s)

_cache: dict = {}


# ----------------------------------------------------------------- launch A
def _build_a(reps=1):
    """QKV projections for a 512-token slice (8-way token-sharded)."""
    nc = bacc.Bacc("TRN2", target_bir_lowering=False, debug=False, num_devices=NC)
    xT = nc.dram_tensor("xT", [128, 8, 512], F16, kind="ExternalInput")
    wq = nc.dram_tensor("wq", [128, 8, D], F16, kind="ExternalInput")
    wk = nc.dram_tensor("wk", [128, 8, D], F16, kind="ExternalInput")
    wv = nc.dram_tensor("wv", [128, 8, D], F16, kind="ExternalInput")
    bqT = nc.dram_tensor("bqT", [128, 8], F32, kind="ExternalInput")
    bkT = nc.dram_tensor("bkT", [128, 8], F32, kind="ExternalInput")
    bv_row = nc.dram_tensor("bv_row", [1, D], F16, kind="ExternalInput")
    qT_o = nc.dram_tensor("qT_o", [128, 8, 512], F16, kind="ExternalOutput")
    kT_o = nc.dram_tensor("kT_o", [128, 8, 512], F16, kind="ExternalOutput")
    v_o = nc.dram_tensor("v_o", [128, 4, D], F16, kind="ExternalOutput")

    from contextlib import nullcontext
    with tile.TileContext(nc) as tc:
        with (tc.For_i(0, reps) if reps > 1 else nullcontext()), \
             tc.tile_pool(name="sg", bufs=1) as sg, \
             tc.tile_pool(name="out", bufs=1) as outp, \
             tc.tile_pool(name="ps", bufs=8, space="PSUM") as ps:
            xt = sg.tile([128, 8, 512], F16, tag="xt")
            nc.sync.dma_start(out=xt[:], in_=xT[:])
            wts = {}
            for nm, dram in (("wq", wq), ("wk", wk), ("wv", wv)):
                wt = sg.tile([128, 8, D], F16, tag=nm)
                nc.sync.dma_start(out=wt[:], in_=dram[:])
                wts[nm] = wt
            bq_sb = sg.tile([128, 8], F32, tag="bq")
            nc.sync.dma_start(out=bq_sb[:], in_=bqT[:])
            bk_sb = sg.tile([128, 8], F32, tag="bk")
            nc.sync.dma_start(out=bk_sb[:], in_=bkT[:])
            bv_sb = sg.tile([1, D], F16, tag="bv")
            nc.sync.dma_start(out=bv_sb[:], in_=bv_row[:])
            ones1 = sg.tile([1, 128], F16, tag="ones1")
            nc.vector.memset(ones1[:], 1.0)

            # Q^T, K^T: out[dout_chunk, t] = W[din, dout].T @ xT[din, t]
            for nm, bias_sb, scale, dst in (
                ("wq", bq_sb, 1.0, qT_o),
                ("wk", bk_sb, 1.0 / 32.0, kT_o),
            ):
                res = outp.tile([128, 8, 512], F16, tag=f"r{nm}")
                for m in range(8):
                    acc = ps.tile([128, 512], F32, tag="acc")
                    for k in range(8):
                        nc.tensor.matmul(
                            acc[:],
                            wts[nm][:, k, m * 128:(m + 1) * 128],
                            xt[:, k, :],
                            start=(k == 0), stop=(k == 7),
                        )
                    nc.scalar.activation(
                        out=res[:, m, :], in_=acc[:], func=AF.Identity,
                        bias=bias_sb[:, m:m + 1], scale=scale,
                    )
                nc.sync.dma_start(out=dst[:], in_=res[:])

            # V natural: out[t_chunk, dout] = xT[din, t_chunk].T @ Wv[din, dout]
            rv = outp.tile([128, 4, D], F16, tag="rv")
            for tcn in range(4):
                for nt in range(2):
                    acc = ps.tile([128, 512], F32, tag="acc")
                    for k in range(8):
                        nc.tensor.matmul(
                            acc[:],
                            xt[:, k, tcn * 128:(tcn + 1) * 128],
                            wts["wv"][:, k, nt * 512:(nt + 1) * 512],
                            start=(k == 0), stop=False,
                        )
                    nc.tensor.matmul(
                        acc[:], ones1[:], bv_sb[:, nt * 512:(nt + 1) * 512],
                        start=False, stop=True,
                    )
                    nc.scalar.activation(
                        out=rv[:, tcn, nt * 512:(nt + 1) * 512], in_=acc[:],
                        func=AF.Copy)
            nc.sync.dma_start(out=v_o[:], in_=rv[:])
    nc.compile()
    return nc


# ----------------------------------------------------------------- launch B
def _chunk_blocks(q):
    """Global 128-row block indices of the two chunks handled by quarter q."""
    return (2 * q, 2 * q + 1), (14 - 2 * q, 15 - 2 * q)


def _build_b(reps=1, stages=5, zdve=True):
    """Uniform attention program (same for all cores).

    Per-core inputs:
      qT [1024, 512] f16 : Q^T, cols = [chunk A 256 | chunk B 256]
      kT [1024, 2048] f16 (pre-scaled 1/32), v [2048, 1024] f16
      wo [1024, 1024] f16, bo_row [1, 1024] f16, ident [128, 128] f16
      masks [24, 128, 256] f16 : per position 0/1 weight-keep masks
      corr [2, 8, 128, 256] f16 : (1/16)*suffix-sum-of-V correction, as
          [chunk, d-pair-chunk, d-within, i-col] added to out^T
    Output: y [512, 1024] f32 (rows = [chunk A | chunk B]).
    """
    nc = bacc.Bacc("TRN2", target_bir_lowering=False, debug=False, num_devices=NC)
    qT = nc.dram_tensor("qT", [128, 8, 512], F16, kind="ExternalInput")
    kT = nc.dram_tensor("kT", [128, 8, T], F16, kind="ExternalInput")
    v = nc.dram_tensor("v", [128, 16, D], F16, kind="ExternalInput")
    wo = nc.dram_tensor("wo", [128, 8, D], F16, kind="ExternalInput")
    bo_row = nc.dram_tensor("bo_row", [1, D], F16, kind="ExternalInput")
    ident = nc.dram_tensor("ident", [128, 128], F16, kind="ExternalInput")
    masks = nc.dram_tensor("masks", [NPOS, 128, 256], F16, kind="ExternalInput")
    corr = nc.dram_tensor("corr", [2, 8, 128, 256], F16, kind="ExternalInput")
    y_o = nc.dram_tensor("y", [512, D], F32, kind="ExternalOutput")

    from contextlib import nullcontext
    with tile.TileContext(nc) as tc:
        with (tc.For_i(0, reps) if reps > 1 else nullcontext()), \
             tc.tile_pool(name="sg", bufs=1) as sg, \
             tc.tile_pool(name="wbuf", bufs=5) as wbuf, \
             tc.tile_pool(name="pt", bufs=5) as ptp, \
             tc.tile_pool(name="rt", bufs=3) as rtp, \
             tc.tile_pool(name="mk", bufs=8) as mkp, \
             tc.tile_pool(name="op", bufs=1) as opp, \
             tc.tile_pool(name="ysb", bufs=2) as ysbp:

            kt = sg.tile([128, 8, T], F16, tag="kt")
            nc.sync.dma_start(out=kt[:], in_=kT[:])
            qt = sg.tile([128, 8, 512], F16, tag="qt")
            nc.sync.dma_start(out=qt[:], in_=qT[:])
            vt = sg.tile([128, 16, D], F16, tag="vt")
            nc.sync.dma_start(out=vt[:], in_=v[:])
            wot = sg.tile([128, 8, D], F16, tag="wot")
            nc.sync.dma_start(out=wot[:], in_=wo[:])
            idt = sg.tile([128, 128], F16, tag="idt")
            nc.sync.dma_start(out=idt[:], in_=ident[:])
            bo_sb = sg.tile([1, D], F16, tag="bo")
            nc.sync.dma_start(out=bo_sb[:], in_=bo_row[:])
            ones1 = sg.tile([1, 128], F16, tag="ones1")
            nc.vector.memset(ones1[:], 1.0)

            # out^T partials per chunk: [128, nsb, 8 pairs, 256]
            outp_tiles = []

            with tc.tile_pool(name="score", bufs=2, space="PSUM") as scp, \
                 tc.tile_pool(name="z", bufs=1 if zdve else 2, space="PSUM") as zp, \
                 tc.tile_pool(name="ot", bufs=2, space="PSUM") as otp:
                for ci in range(2):
                    npos = CNT[ci]
                    coff = ci * 256
                    poff = 0 if ci == 0 else CNT[0]   # mask index offset
                    nsb = npos // 4                   # super-blocks of 2 pairs
                    outp_c = opp.tile([128, nsb, 8, 256], F16, tag=f"outp{ci}")
                    outp_tiles.append((outp_c, nsb))

                    for s in range(nsb):
                        wts_s = {}
                        # ---- phase 1 per jb position: scores -> exp -> Z -> w
                        for half in range(4):
                            jb = s * 4 + half
                            wt = wbuf.tile([128, 16, 256], F16, tag="w")
                            zt = None if zdve else zp.tile([128, 256], F32, tag="z")
                            pts = []
                            for g in range(4):
                                sc = scp.tile([128, 4, 256], F32, tag="sc")
                                # slot order puts row-group-0 heads in bank 0
                                # and row-group-64 heads in bank 1: concurrent
                                # different-row-group matmuls must not share a
                                # PSUM bank (HW constraint, not in CoreSim).
                                for hh, h in enumerate(_GRP_HEADS[g]):
                                    c, off = h // 2, (h % 2) * 64
                                    nc.tensor.matmul(
                                        sc[:, hh, :],
                                        kt[off:off + 64, c,
                                           jb * 128:(jb + 1) * 128],
                                        qt[off:off + 64, c, coff:coff + 256],
                                        start=True, stop=True,
                                        tile_position=(off, 0),
                                    )
                                pt = ptp.tile([128, 4, 256], F16, tag="p")
                                nc.scalar.activation(out=pt[:], in_=sc[:],
                                                     func=AF.Exp)
                                pts.append(pt)
                                if stages >= 2 and not zdve:
                                    for hh in range(4):
                                        nc.tensor.matmul(
                                            zt[:], idt[:], pt[:, hh, :],
                                            start=(g == 0 and hh == 0),
                                            stop=(g == 3 and hh == 3),
                                        )
                            if stages >= 3:
                                r32 = rtp.tile([128, 256], F32, tag="r32")
                                if zdve:
                                    t01 = ptp.tile([128, 4, 256], F16, tag="t01")
                                    t23 = ptp.tile([128, 4, 256], F16, tag="t23")
                                    nc.vector.tensor_add(t01[:], pts[0][:], pts[1][:])
                                    nc.vector.tensor_add(t23[:], pts[2][:], pts[3][:])
                                    nc.vector.tensor_add(t01[:], t01[:], t23[:])
                                    u2 = rtp.tile([128, 2, 256], F16, tag="u2")
                                    nc.vector.tensor_add(u2[:], t01[:, 0:2, :], t01[:, 2:4, :])
                                    z32 = rtp.tile([128, 256], F32, tag="z32")
                                    nc.vector.tensor_add(z32[:], u2[:, 0, :], u2[:, 1, :])
                                    nc.vector.reciprocal_approx_fast(out=r32[:], in_=z32[:])
                                else:
                                    nc.vector.reciprocal_approx_fast(out=r32[:], in_=zt[:])
                                mk = mkp.tile([128, 256], F16, tag="mk")
                                nc.sync.dma_start(out=mk[:], in_=masks[poff + jb, :, :])
                                r16 = rtp.tile([128, 256], F16, tag="r16")
                                nc.vector.tensor_mul(r16[:], r32[:], mk[:])
                                rb = r16[:].rearrange("p (a f) -> p a f", a=1) \
                                           .to_broadcast([128, 4, 256])
                                for g in range(4):
                                    nc.vector.tensor_mul(
                                        wt[:, 4 * g:4 * g + 4, :], pts[g][:], rb)
                            wts_s[half] = wt
                        # ---- phase 2: WV matmuls (col-packed head pairs)
                        for pr in range(8 if stages >= 4 else 0):
                            ops_ = otp.tile([128, 256], F32, tag="ot")
                            for sub in range(2):
                                h = 2 * pr + sub
                                po = sub * 64
                                g_, s_ = _HEAD_SLOT[h]
                                for half in range(4):
                                    nc.tensor.matmul(
                                        ops_[po:po + 64, :],
                                        vt[:, s * 4 + half, h * 64:(h + 1) * 64],
                                        wts_s[half][:, 4 * g_ + s_, :],
                                        start=(half == 0), stop=(half == 3),
                                        tile_position=(0, po),
                                    )
                            nc.any.tensor_copy(outp_c[:, s, pr, :], ops_[:])

            # ---- s-reduction + suffix correction (DVE) + output projection
            with tc.tile_pool(name="yps", bufs=2, space="PSUM") as yps, \
                 tc.tile_pool(name="ck", bufs=4) as ckp:
                for ci in range(2 if stages >= 4 else 0):
                    outp_c, nsb = outp_tiles[ci]
                    for pr in range(8):
                        ck = ckp.tile([128, 256], F16, tag="ck")
                        nc.sync.dma_start(out=ck[:], in_=corr[ci, pr, :, :])
                        nc.vector.tensor_add(
                            outp_c[:, 0, pr, :], outp_c[:, 0, pr, :], ck[:])
                        for s in range(1, nsb):
                            nc.vector.tensor_add(
                                outp_c[:, 0, pr, :], outp_c[:, 0, pr, :],
                                outp_c[:, s, pr, :])
                    for ib in range(2 if stages >= 5 else 0):
                        for nt in range(2):
                            acc = yps.tile([128, 512], F32, tag="yacc")
                            for dc in range(8):
                                nc.tensor.matmul(
                                    acc[:],
                                    outp_c[:, 0, dc, ib * 128:(ib + 1) * 128],
                                    wot[:, dc, nt * 512:(nt + 1) * 512],
                                    start=(dc == 0), stop=False,
                                )
                            nc.tensor.matmul(
                                acc[:], ones1[:], bo_sb[:, nt * 512:(nt + 1) * 512],
                                start=False, stop=True,
                            )
                            yt = ysbp.tile([128, 512], F32, tag="yt")
                            nc.vector.tensor_copy(yt[:], acc[:])
                            nc.sync.dma_start(
                                out=y_o[(ci * 2 + ib) * 128:(ci * 2 + ib + 1) * 128,
                                        nt * 512:(nt + 1) * 512],
                                in_=yt[:])
    nc.compile()
    return nc


# ------------------------------------------------------------------- driver
def _masks_for(q):
    """[24, 128, 256] keep-masks for quarter q (padding positions -> 0)."""
    i = np.arange(128)
    tri = (i[:, None] <= i[None, :]).astype(np.float32)   # [j, i], keep j<=i
    ones = np.ones((128, 128), np.float32)
    zeros = np.zeros((128, 128), np.float32)
    out = np.zeros((NPOS, 128, 256), np.float32)
    for ci, (b0, b1) in enumerate(_chunk_blocks(q)):
        cnt_real = b1 + 1                  # real causal j-blocks
        poff = 0 if ci == 0 else CNT[0]
        for p in range(CNT[ci]):
            if p >= cnt_real:
                continue                   # padding: stays zero
            left = tri if p == b0 else (ones if p < b0 else zeros)
            right = tri if p == b1 else (ones if p < b1 else zeros)
            out[poff + p] = np.concatenate([left, right], axis=1)
    return out.astype(np.float16)


def kernel(x, Wq, bq, Wk, bk, Wv, bv, Wo, bo):
    x = np.asarray(x, dtype=np.float32)
    Wq, bq = np.asarray(Wq, np.float32), np.asarray(bq, np.float32)
    Wk, bk = np.asarray(Wk, np.float32), np.asarray(bk, np.float32)
    Wv, bv = np.asarray(Wv, np.float32), np.asarray(bv, np.float32)
    Wo, bo = np.asarray(Wo, np.float32), np.asarray(bo, np.float32)

    if "a" not in _cache:
        _cache["a"] = _build_a()
    if "b" not in _cache:
        _cache["b"] = _build_b()

    def part8(a):  # [1024, N] -> [128, 8, N] partition-major contiguous
        return np.ascontiguousarray(a.reshape(8, 128, -1).transpose(1, 0, 2))

    x_flat = x.reshape(B * T, D)
    wq16, wk16, wv16 = (part8(w.astype(np.float16)) for w in (Wq, Wk, Wv))
    bqT = np.ascontiguousarray(bq.reshape(8, 128).T).astype(np.float32)
    bkT = np.ascontiguousarray((bk / 32.0).reshape(8, 128).T).astype(np.float32)
    bv_row = bv.astype(np.float16)[None, :]
    in_maps_a = []
    for c in range(NC):
        xTs = part8(np.ascontiguousarray(x_flat[c * 512:(c + 1) * 512].T).astype(np.float16))
        in_maps_a.append(dict(xT=xTs, wq=wq16, wk=wk16, wv=wv16,
                              bqT=bqT, bkT=bkT, bv_row=bv_row))
    res_a = run_bass_kernel_spmd(_cache["a"], in_maps_a, core_ids=list(range(NC)))

    def unpart(a):  # [128, C, N] -> [128*C, N]
        return a.transpose(1, 0, 2).reshape(-1, a.shape[2])

    qT_full = [np.concatenate([unpart(res_a.results[b_ * 4 + i]["qT_o"])
                               for i in range(4)], axis=1) for b_ in range(B)]
    kT_full = [np.concatenate([unpart(res_a.results[b_ * 4 + i]["kT_o"])
                               for i in range(4)], axis=1) for b_ in range(B)]
    v_full = [np.concatenate([unpart(res_a.results[b_ * 4 + i]["v_o"])
                              for i in range(4)], axis=0) for b_ in range(B)]

    ident = np.eye(128, dtype=np.float16)
    bo_row = bo.astype(np.float16)[None, :]
    wo16 = part8(Wo.astype(np.float16))
    masks_q = [_masks_for(q) for q in range(4)]

    in_maps_b = []
    for c in range(NC):
        b_, qq = c // 4, c % 4
        (a0, _a1), (b0, _b1) = _chunk_blocks(qq)
        qT_core = np.ascontiguousarray(np.concatenate(
            [qT_full[b_][:, a0 * 128:a0 * 128 + 256],
             qT_full[b_][:, b0 * 128:b0 * 128 + 256]], axis=1))
        vf32 = v_full[b_].astype(np.float32)
        suffix = (vf32.sum(0)[None, :] - np.cumsum(vf32, axis=0)) / 16.0  # [T, D]
        corr = np.zeros((2, 8, 128, 256), np.float32)
        for ci, cblk in enumerate((a0, b0)):
            blk = suffix[cblk * 128: cblk * 128 + 256]          # [256 i, 1024 d]
            corr[ci] = blk.T.reshape(8, 128, 256)
        v16 = np.ascontiguousarray(
            v_full[b_].reshape(16, 128, D).transpose(1, 0, 2))
        in_maps_b.append(dict(
            qT=part8(qT_core), kT=part8(np.ascontiguousarray(kT_full[b_])),
            v=v16, wo=wo16, bo_row=bo_row,
            ident=ident, masks=masks_q[qq], corr=corr.astype(np.float16)))

    res_b = run_bass_kernel_spmd(_cache["b"], in_maps_b, core_ids=list(range(NC)))

    y = np.zeros((B, T, D), np.float32)
    for c in range(NC):
        b_, qq = c // 4, c % 4
        (a0, _), (b0, _) = _chunk_blocks(qq)
        yc = res_b.results[c]["y"]
        y[b_, a0 * 128:a0 * 128 + 256] = yc[:256]
        y[b_, b0 * 128:b0 * 128 + 256] = yc[256:]
    return y



# revision 30
# speedup vs baseline: 1.8791x; 1.8791x over previous
"""Trainium2 Bass kernel for nn_MultiHeadAttention (softmax over HEAD axis).

Problem: B=2, T=2048, D=1024, H=16, HD=64.
  Q,K,V = x@W* + b*;  score = QK^T/32 with causal positions set to -1e10
  weight = softmax(score, axis=HEADS)  -> masked (j>i) entries get exactly 1/16
  out = weight@V;  y = out@Wo + bo

Exact identity used: for row i,
  out_h[i] = sum_{j<=i} w_h[i,j] V_h[j] + (1/16) sum_{j>i} V_h[j]
where w is the head-softmax of unmasked scores. Weights are computed only on
causal j-blocks (0/1 masks zero the diagonal-block upper triangle), and the
(1/16)*suffix-sum(V) correction is a host-precomputed additive matrix.

Sharding (8 cores, two launches):
  Launch A: QKV projections, 8-way token-sharded.
  Launch B: attention + out-proj. Core c = (batch c//4, quarter q=c%4).
    Quarter q owns 4 i-blocks (128 rows each): the mirror pairs
    {2q, 15-2q} + {2q+1, 14-2q}; total causal work = 34 j-block positions
    per core, identical on every core. The uniform SPMD program runs 7
    slots of 128 i-columns with capacities (10,9,5,4,3,2,1) = 34 positions
    of [128 j x 128 i]; the HOST assigns which (i-block, j-range) each slot
    processes per core (kT/V are packed per-position, Q^T per-slot, masks/
    corr per-position/slot), so there is ZERO padding. i-blocks split
    across slots produce partial y rows that the host sums (minus the
    duplicated bias).

All matmul inputs fp16 (1 cyc/row on PE), accumulation fp32 in PSUM.
WV accumulates in PSUM across all positions of a slot; softmax runs on
ACT (exp) + DVE (Z tree, recip, normalize) + Pool (one add offloaded).
"""

import numpy as np

import concourse.bass as bass
import concourse.tile as tile
from concourse import bacc, mybir
from concourse.bass_utils import run_bass_kernel_spmd

F16 = mybir.dt.float16
F32 = mybir.dt.float32
AF = mybir.ActivationFunctionType

B, T, D, H, HD = 2, 2048, 1024, 16, 64
NC = 8
NBLK = T // 128          # 16

# ---- slot schedule: 7 slots x 128 i-cols, capacities sum to 34 ----------
CAPS = (10, 9, 5, 4, 3, 2, 1)
NSLOT = len(CAPS)
NPOS = sum(CAPS)         # 34
START = [sum(CAPS[:s]) for s in range(NSLOT)]

# per quarter: (slot, i-block, first j-block) for each slot
SLOT_MAP = {
 0: [(0, 14, 0), (1, 15, 0), (2, 14, 10), (3, 15, 9), (4, 15, 13), (5, 1, 0), (6, 0, 0)],
 1: [(0, 12, 0), (1, 13, 0), (2, 13, 9), (3, 3, 0), (4, 12, 10), (5, 2, 0), (6, 2, 2)],
 2: [(0, 10, 0), (1, 11, 0), (2, 4, 0), (3, 5, 0), (4, 11, 9), (5, 5, 4), (6, 10, 10)],
 3: [(0, 9, 0), (1, 8, 0), (2, 7, 0), (3, 6, 0), (4, 7, 5), (5, 6, 4), (6, 6, 6)],
}

# Score matmuls contract over the full 128 partitions with the unused
# 64-row half of each per-head K strip zero-padded (kTp is stored per-head).
# This avoids PE row-group tiling entirely, which is required because
# row-tiled matmuls interleaved inside the open col-tiled WV accumulation
# groups corrupt PSUM on hardware.
_GRP_HEADS = [[0, 1, 2, 3], [4, 5, 6, 7], [8, 9, 10, 11], [12, 13, 14, 15]]
_HEAD_SLOT = {}
for _g in range(4):
    for _s, _h in enumerate(_GRP_HEADS[_g]):
        _HEAD_SLOT[_h] = (_g, _s)

_cache: dict = {}


# ----------------------------------------------------------------- launch A
def _build_a(reps=1):
    """QKV projections for a 512-token slice (8-way token-sharded)."""
    nc = bacc.Bacc("TRN2", target_bir_lowering=False, debug=False, num_devices=NC)
    xT = nc.dram_tensor("xT", [128, 8, 512], F16, kind="ExternalInput")
    wq = nc.dram_tensor("wq", [128, 8, D], F16, kind="ExternalInput")
    wk = nc.dram_tensor("wk", [128, 8, D], F16, kind="ExternalInput")
    wv = nc.dram_tensor("wv", [128, 8, D], F16, kind="ExternalInput")
    bqT = nc.dram_tensor("bqT", [128, 8], F32, kind="ExternalInput")
    bkT = nc.dram_tensor("bkT", [128, 8], F32, kind="ExternalInput")
    bv_row = nc.dram_tensor("bv_row", [1, D], F16, kind="ExternalInput")
    qT_o = nc.dram_tensor("qT_o", [128, 8, 512], F16, kind="ExternalOutput")
    kT_o = nc.dram_tensor("kT_o", [128, 8, 512], F16, kind="ExternalOutput")
    v_o = nc.dram_tensor("v_o", [128, 4, D], F16, kind="ExternalOutput")

    from contextlib import nullcontext
    with tile.TileContext(nc) as tc:
        with (tc.For_i(0, reps) if reps > 1 else nullcontext()), \
             tc.tile_pool(name="sg", bufs=1) as sg, \
             tc.tile_pool(name="out", bufs=1) as outp, \
             tc.tile_pool(name="ps", bufs=8, space="PSUM") as ps:
            xt = sg.tile([128, 8, 512], F16, tag="xt")
            nc.sync.dma_start(out=xt[:], in_=xT[:])
            wts = {}
            for nm, dram in (("wq", wq), ("wk", wk), ("wv", wv)):
                wts[nm] = sg.tile([128, 8, D], F16, tag=nm, name=nm)
            # quarter-granularity weight streaming in consumption order
            nc.sync.dma_start(out=wts["wq"][:, :, 0:256], in_=wq[:, :, 0:256])
            bq_sb = sg.tile([128, 8], F32, tag="bq")
            nc.sync.dma_start(out=bq_sb[:], in_=bqT[:])
            bk_sb = sg.tile([128, 8], F32, tag="bk")
            nc.sync.dma_start(out=bk_sb[:], in_=bkT[:])
            bv_sb = sg.tile([1, D], F16, tag="bv")
            nc.sync.dma_start(out=bv_sb[:], in_=bv_row[:])
            for lo, hi in ((256, 512), (512, 768), (768, 1024)):
                nc.sync.dma_start(out=wts["wq"][:, :, lo:hi], in_=wq[:, :, lo:hi])
            for lo, hi in ((0, 256), (256, 512), (512, 768), (768, 1024)):
                nc.sync.dma_start(out=wts["wk"][:, :, lo:hi], in_=wk[:, :, lo:hi])
            for lo, hi in ((0, 256), (256, 512), (512, 768), (768, 1024)):
                nc.sync.dma_start(out=wts["wv"][:, :, lo:hi], in_=wv[:, :, lo:hi])
            ones1 = sg.tile([1, 128], F16, tag="ones1")
            nc.vector.memset(ones1[:], 1.0)

            # Q^T, K^T: out[dout_chunk, t] = W[din, dout].T @ xT[din, t]
            for nm, bias_sb, scale, dst in (
                ("wq", bq_sb, 1.0, qT_o),
                ("wk", bk_sb, 1.0 / 32.0, kT_o),
            ):
                res = outp.tile([128, 8, 512], F16, tag=f"r{nm}", name=f"r{nm}")
                for m in range(8):
                    acc = ps.tile([128, 512], F32, tag="acc")
                    for k in range(8):
                        nc.tensor.matmul(
                            acc[:],
                            wts[nm][:, k, m * 128:(m + 1) * 128],
                            xt[:, k, :],
                            start=(k == 0), stop=(k == 7),
                        )
                    nc.scalar.activation(
                        out=res[:, m, :], in_=acc[:], func=AF.Identity,
                        bias=bias_sb[:, m:m + 1], scale=scale,
                    )
                    if m % 2 == 1:
                        nc.sync.dma_start(out=dst[:, m - 1:m + 1, :],
                                          in_=res[:, m - 1:m + 1, :])

            # V natural: out[t_chunk, dout] = xT[din, t_chunk].T @ Wv[din, dout]
            rv = outp.tile([128, 4, D], F16, tag="rv")
            for tcn in range(4):
                for nt in range(2):
                    acc = ps.tile([128, 512], F32, tag="acc")
                    for k in range(8):
                        nc.tensor.matmul(
                            acc[:],
                            xt[:, k, tcn * 128:(tcn + 1) * 128],
                            wts["wv"][:, k, nt * 512:(nt + 1) * 512],
                            start=(k == 0), stop=False,
                        )
                    nc.tensor.matmul(
                        acc[:], ones1[:], bv_sb[:, nt * 512:(nt + 1) * 512],
                        start=False, stop=True,
                    )
                    nc.scalar.activation(
                        out=rv[:, tcn, nt * 512:(nt + 1) * 512], in_=acc[:],
                        func=AF.Copy)
                nc.sync.dma_start(out=v_o[:, tcn, :], in_=rv[:, tcn, :])
    nc.compile()
    return nc


# ----------------------------------------------------------------- launch B
def _build_b(reps=1, pool_zb=False, pool_wm='lag', drain=6, order=(0, 6, 1, 5, 2, 4, 3)):
    """Uniform attention program (same for all cores), v3: 7-slot schedule.

    Per-core inputs (all host-packed per the core's SLOT_MAP):
      qTs [128, 8, 896] f16   : Q^T slot columns (slot s -> its i-block)
      kTp [34, 128, 8, 128]   : K^T per position, packed j-blocks (x 1/32)
      vp  [34, 128, 1024]     : V per position, packed j-blocks
      wo  [128, 8, 1024] f16, bo_row [1, 1024] f16
      masks [128, 34, 128] f16: per-position weight-keep masks
      corr [128, 7, 8, 128] f16 : suffix-correction per slot (zeros on
          slots that are not the designated carrier of their i-block)
    Output: y [896, 1024] f32 (7 slots x 128 rows; host merges split slots).
    """
    nc = bacc.Bacc("TRN2", target_bir_lowering=False, debug=False, num_devices=NC)
    qTs = nc.dram_tensor("qTs", [128, 8, NSLOT * 128], F16, kind="ExternalInput")
    kTp = nc.dram_tensor("kTp", [NPOS, 128, 16, 128], F16, kind="ExternalInput")
    vp = nc.dram_tensor("vp", [NPOS, 128, D], F16, kind="ExternalInput")
    wo = nc.dram_tensor("wo", [128, 8, D], F16, kind="ExternalInput")
    bo_row = nc.dram_tensor("bo_row", [1, D], F16, kind="ExternalInput")
    masks = nc.dram_tensor("masks", [128, NPOS, 128], F16, kind="ExternalInput")
    corr = nc.dram_tensor("corr", [128, NSLOT, 8, 128], F16, kind="ExternalInput")
    ident = nc.dram_tensor("ident", [128, 128], F16, kind="ExternalInput")
    y_o = nc.dram_tensor("y", [NSLOT * 128, D], F32, kind="ExternalOutput")

    from contextlib import nullcontext
    with tile.TileContext(nc) as tc:
        with (tc.For_i(0, reps) if reps > 1 else nullcontext()), \
             tc.tile_pool(name="sg", bufs=1) as sg, \
             tc.tile_pool(name="ktp", bufs=4) as ktpool, \
             tc.tile_pool(name="vtp", bufs=16) as vtpool, \
             tc.tile_pool(name="wbuf", bufs=13) as wbuf, \
             tc.tile_pool(name="pt", bufs=3) as ptp, \
             tc.tile_pool(name="zt", bufs=2) as ztp, \
             tc.tile_pool(name="rt", bufs=3) as rtp, \
             tc.tile_pool(name="op", bufs=1) as opp, \
             tc.tile_pool(name="ysb", bufs=2) as ysbp:

            # ---- input DMAs: qt slot 0 + first k/v positions first ----
            qt = sg.tile([128, 8, NSLOT * 128], F16, tag="qt")
            q0 = (order or range(NSLOT))[0]
            nc.sync.dma_start(out=qt[:, :, q0 * 128:(q0 + 1) * 128],
                              in_=qTs[:, :, q0 * 128:(q0 + 1) * 128])
            kts, vts = {}, {}

            def fetch(gp):
                kts[gp] = ktpool.tile([128, 16, 128], F16, tag="kt", name="ktt")
                nc.sync.dma_start(out=kts[gp][:], in_=kTp[gp, :, :, :])
                vts[gp] = vtpool.tile([128, D], F16, tag="vt", name="vtt")
                nc.sync.dma_start(out=vts[gp][:], in_=vp[gp, :, :])

            seq = []
            for s in (order or range(NSLOT)):
                for p in range(CAPS[s]):
                    seq.append((s, p, START[s] + p))
            fetch(seq[0][2])
            mk_sb = sg.tile([128, NPOS, 128], F16, tag="mk")
            g0_, g1_ = seq[0][2], seq[1][2]
            nc.sync.dma_start(out=mk_sb[:, g0_, :], in_=masks[:, g0_, :])
            nc.sync.dma_start(out=mk_sb[:, g1_, :], in_=masks[:, g1_, :])
            fetch(seq[1][2])
            idt = sg.tile([128, 128], F16, tag="idt")
            nc.sync.dma_start(out=idt[:], in_=ident[:])
            corr_sb = sg.tile([128, NSLOT, 8, 128], F16, tag="corr")
            s0_ = (order or range(NSLOT))[0]
            nc.sync.dma_start(out=corr_sb[:, s0_, :, :], in_=corr[:, s0_, :, :])
            fetch(seq[2][2])
            mk_done = sorted([g0_, g1_])
            rest = [g for g in range(NPOS) if g not in mk_done]
            # contiguous runs of remaining mask columns, one DMA each
            runs = []
            for g in rest:
                if runs and runs[-1][1] == g:
                    runs[-1][1] = g + 1
                else:
                    runs.append([g, g + 1])
            for a, bnd in runs:
                nc.sync.dma_start(out=mk_sb[:, a:bnd, :], in_=masks[:, a:bnd, :])
            fetch(seq[3][2])
            for s_ in (order or range(NSLOT)):
                if s_ != s0_:
                    nc.sync.dma_start(out=corr_sb[:, s_, :, :],
                                      in_=corr[:, s_, :, :])
            for i, s in enumerate(order or range(NSLOT)):
                if i:
                    nc.sync.dma_start(out=qt[:, :, s * 128:(s + 1) * 128],
                                      in_=qTs[:, :, s * 128:(s + 1) * 128])
            bo_sb = sg.tile([1, D], F16, tag="bo")
            nc.sync.dma_start(out=bo_sb[:], in_=bo_row[:])
            ones1 = sg.tile([1, 128], F16, tag="ones1")
            nc.vector.memset(ones1[:], 1.0)
            wot = sg.tile([128, 8, D], F16, tag="wot")
            wot_dma = [False]
            next_fetch = [4]

            outT = [opp.tile([128, 8, 128], F16, tag=f"outT{s}", name=f"outT{s}")
                    for s in range(NSLOT)]

            stash = []   # pending pool wmul: (wt, pt, rb)

            def flush_pool_wm():
                if stash:
                    wt_, pt_, rb_ = stash.pop()
                    nc.gpsimd.tensor_mul(wt_[:, 12:16, :], pt_[:, 12:16, :], rb_)

            def softmax_w(s, gp):
                """scores+exp+Z+w for global position gp in slot s."""
                pt = ptp.tile([128, 16, 128], F16, tag="pt")
                for g in range(4):
                    sc = scp.tile([128, 4, 128], F32, tag="sc", name="sc")
                    for hh, h in enumerate(_GRP_HEADS[g]):
                        c = h // 2
                        nc.tensor.matmul(
                            sc[:, hh, :],
                            kts[gp][:, h, :],
                            qt[:, c, s * 128:(s + 1) * 128],
                            start=True, stop=True,
                        )
                    nc.scalar.activation(out=pt[:, 4 * g:4 * g + 4, :],
                                         in_=sc[:], func=AF.Exp)
                flush_pool_wm()
                # Z tree: one 512-el add on Pool, rest DVE
                za = ztp.tile([128, 4, 128], F16, tag="za")
                nc.vector.tensor_add(za[:], pt[:, 0:4, :], pt[:, 8:12, :])
                zb = ztp.tile([128, 4, 128], F16, tag="zb")
                (nc.gpsimd if pool_zb else nc.vector).tensor_add(
                    zb[:], pt[:, 4:8, :], pt[:, 12:16, :])
                nc.vector.tensor_add(za[:], za[:], zb[:])
                zu = rtp.tile([128, 2, 128], F16, tag="zu")
                nc.vector.tensor_add(zu[:], za[:, 0:2, :], za[:, 2:4, :])
                z32 = rtp.tile([128, 128], F32, tag="z32")
                nc.vector.tensor_add(z32[:], zu[:, 0, :], zu[:, 1, :])
                r32 = rtp.tile([128, 128], F32, tag="r32")
                nc.vector.reciprocal_approx_fast(out=r32[:], in_=z32[:])
                r16 = rtp.tile([128, 128], F16, tag="r16")
                nc.vector.tensor_mul(r16[:], r32[:], mk_sb[:, gp, :])
                rb = r16[:].rearrange("p (a f) -> p a f", a=1) \
                           .to_broadcast([128, 4, 128])
                wt = wbuf.tile([128, 16, 128], F16, tag="w")
                for g in range(3):
                    nc.vector.tensor_mul(wt[:, 4 * g:4 * g + 4, :],
                                         pt[:, 4 * g:4 * g + 4, :], rb)
                if pool_wm == 'lag':
                    stash.append((wt, pt, rb))
                elif pool_wm == 'now':
                    nc.gpsimd.tensor_mul(wt[:, 12:16, :], pt[:, 12:16, :], rb)
                else:
                    nc.vector.tensor_mul(wt[:, 12:16, :], pt[:, 12:16, :], rb)
                return wt

            wts_store = {}

            def chain(s, oi, pr, sub):
                """One WV accumulation region: contiguous open->close group,
                seeded with the suffix correction (half per col group)."""
                po = sub * 64
                h = 2 * pr + sub
                g_, s_ = _HEAD_SLOT[h]
                cap = CAPS[s]
                for p in range(cap):
                    gp = START[s] + p
                    nc.tensor.matmul(
                        ots[oi % 2][po:po + 64, pr, :],
                        vts[gp][:, h * 64:(h + 1) * 64],
                        wts_store[gp][:, 4 * g_ + s_, :],
                        start=(p == 0), stop=(p == cap - 1),
                        tile_position=(0, po),
                    )

            def copies(s, oi):
                for pr in range(8):
                    nc.vector.tensor_add(outT[s][:, pr, :],
                                         ots[oi % 2][:, pr, :],
                                         corr_sb[:, s, pr, :])

            def outproj(s, nts=(0, 1)):
                for nt in nts:
                    acc = yps.tile([128, 512], F32, tag="yacc")
                    for dc in range(8):
                        nc.tensor.matmul(
                            acc[:],
                            outT[s][:, dc, :],
                            wot[:, dc, nt * 512:(nt + 1) * 512],
                            start=(dc == 0), stop=False,
                        )
                    nc.tensor.matmul(
                        acc[:], ones1[:], bo_sb[:, nt * 512:(nt + 1) * 512],
                        start=False, stop=True,
                    )
                    yt = ysbp.tile([128, 512], F32, tag="yt")
                    nc.scalar.copy(yt[:], acc[:])
                    nc.sync.dma_start(
                        out=y_o[s * 128:(s + 1) * 128,
                                nt * 512:(nt + 1) * 512],
                        in_=yt[:])

            with tc.tile_pool(name="score", bufs=3, space="PSUM") as scp, \
                 tc.tile_pool(name="ot", bufs=2, space="PSUM") as otp, \
                 tc.tile_pool(name="yps", bufs=1, space="PSUM") as yps:
                ots = [otp.tile([128, 8, 128], F32, tag="ot", name=f"ot{i}")
                       for i in range(2)]
                slot_oi = {s: i for i, s in enumerate(order)}
                cq = []               # chain FIFO: (s, oi, pr, sub)
                left = {}             # slot -> chains not yet emitted
                tasks = []            # [(due_si, slot, nt)]

                def drain_chains(si, n):
                    while cq and n > 0:
                        s_, oi_, pr_, sub_ = cq.pop(0)
                        chain(s_, oi_, pr_, sub_)
                        left[s_] -= 1
                        if left[s_] == 0:
                            copies(s_, oi_)
                            tasks.append((si + 1, s_, 0))
                            tasks.append((si + 3, s_, 1))
                        n -= 1

                for si, (s, p, gp) in enumerate(seq):
                    drain_chains(si, drain)
                    wts_store[gp] = softmax_w(s, gp)
                    if p == CAPS[s] - 1:
                        flush_pool_wm()
                        oi = slot_oi[s]
                        left[s] = 16
                        cq.extend((s, oi, pr, sub)
                                  for pr in range(8) for sub in (0, 1))
                    if next_fetch[0] < NPOS:
                        fetch(seq[next_fetch[0]][2])
                        next_fetch[0] += 1
                    if cq and not wot_dma[0]:
                        nc.sync.dma_start(out=wot[:], in_=wo[:])
                        wot_dma[0] = True
                    while tasks and tasks[0][0] <= si:
                        _, sl_, nt_ = tasks.pop(0)
                        outproj(sl_, (nt_,))
                drain_chains(NPOS, 10 ** 9)
                for _, sl_, nt_ in tasks:
                    outproj(sl_, (nt_,))
    nc.compile()
    return nc


# ------------------------------------------------------------------- driver
def _pack_b(qT_full, kT_full, v_full, wo16, bo_row, q):
    """Build launch-B inputs for quarter q from full per-batch matrices.

    qT_full [1024, 2048] f16 (Q^T), kT_full [1024, 2048] f16 (pre-scaled),
    v_full [2048, 1024] f16, wo16 [128, 8, 1024] f16, bo_row [1, 1024] f16.
    """
    def part8(a):
        return np.ascontiguousarray(a.reshape(8, 128, -1).transpose(1, 0, 2))

    segs = sorted(SLOT_MAP[q])
    qTs = np.zeros((128, 8, NSLOT * 128), np.float16)
    kTp = np.zeros((NPOS, 128, 16, 128), np.float16)
    vpp = np.zeros((NPOS, 128, D), np.float16)
    mk = np.zeros((128, NPOS, 128), np.float16)
    tri = (np.arange(128)[:, None] <= np.arange(128)[None, :])  # keep j<=i

    vf32 = v_full.astype(np.float32)
    suffix = (vf32.sum(0)[None, :] - np.cumsum(vf32, axis=0)) / 16.0  # [T, D]
    corr = np.zeros((128, NSLOT, 8, 128), np.float16)
    carrier = {}   # i-block -> first slot holding it
    for s, ib, j0 in segs:
        qTs[:, :, s * 128:(s + 1) * 128] = part8(
            np.ascontiguousarray(qT_full[:, ib * 128:(ib + 1) * 128]))
        for p in range(CAPS[s]):
            gp = START[s] + p
            jj = j0 + p
            k8 = part8(np.ascontiguousarray(
                kT_full[:, jj * 128:(jj + 1) * 128]))    # [128, 8, 128]
            for h in range(16):
                off = (h % 2) * 64
                kTp[gp, off:off + 64, h, :] = k8[off:off + 64, h // 2, :]
            vpp[gp] = v_full[jj * 128:(jj + 1) * 128, :]
            mk[:, gp, :] = tri if jj == ib else 1.0
        if ib not in carrier:
            carrier[ib] = s
            blk = suffix[ib * 128:(ib + 1) * 128]      # [128 i, 1024 d]
            corr[:, s, :, :] = blk.T.reshape(8, 128, 128).transpose(1, 0, 2)
    return dict(qTs=np.ascontiguousarray(qTs),
                kTp=np.ascontiguousarray(kTp),
                vp=np.ascontiguousarray(vpp), wo=wo16, bo_row=bo_row,
                masks=np.ascontiguousarray(mk),
                corr=np.ascontiguousarray(corr),
                ident=np.eye(128, dtype=np.float16))


def _unpack_y(y_slots, bo, q):
    """y_slots [896, 1024] -> per-block rows dict {i-block: [128, 1024]}."""
    segs = sorted(SLOT_MAP[q])
    acc = {}
    nslots = {}
    for s, ib, _ in segs:
        part = y_slots[s * 128:(s + 1) * 128].astype(np.float32)
        if ib in acc:
            acc[ib] = acc[ib] + part
            nslots[ib] += 1
        else:
            acc[ib] = part.copy()
            nslots[ib] = 1
    for ib in acc:
        if nslots[ib] > 1:
            acc[ib] -= (nslots[ib] - 1) * bo[None, :]
    return acc


def kernel(x, Wq, bq, Wk, bk, Wv, bv, Wo, bo):
    x = np.asarray(x, dtype=np.float32)
    Wq, bq = np.asarray(Wq, np.float32), np.asarray(bq, np.float32)
    Wk, bk = np.asarray(Wk, np.float32), np.asarray(bk, np.float32)
    Wv, bv = np.asarray(Wv, np.float32), np.asarray(bv, np.float32)
    Wo, bo = np.asarray(Wo, np.float32), np.asarray(bo, np.float32)

    if "a" not in _cache:
        _cache["a"] = _build_a()
    if "b" not in _cache:
        _cache["b"] = _build_b()

    def part8(a):  # [1024, N] -> [128, 8, N] partition-major contiguous
        return np.ascontiguousarray(a.reshape(8, 128, -1).transpose(1, 0, 2))

    x_flat = x.reshape(B * T, D)
    wq16, wk16, wv16 = (part8(w.astype(np.float16)) for w in (Wq, Wk, Wv))
    bqT = np.ascontiguousarray(bq.reshape(8, 128).T).astype(np.float32)
    bkT = np.ascontiguousarray((bk / 32.0).reshape(8, 128).T).astype(np.float32)
    bv_row = bv.astype(np.float16)[None, :]
    in_maps_a = []
    for c in range(NC):
        xTs = part8(np.ascontiguousarray(x_flat[c * 512:(c + 1) * 512].T).astype(np.float16))
        in_maps_a.append(dict(xT=xTs, wq=wq16, wk=wk16, wv=wv16,
                              bqT=bqT, bkT=bkT, bv_row=bv_row))
    res_a = run_bass_kernel_spmd(_cache["a"], in_maps_a, core_ids=list(range(NC)))

    def unpart(a):  # [128, C, N] -> [128*C, N]
        return a.transpose(1, 0, 2).reshape(-1, a.shape[2])

    qT_full = [np.concatenate([unpart(res_a.results[b_ * 4 + i]["qT_o"])
                               for i in range(4)], axis=1) for b_ in range(B)]
    kT_full = [np.concatenate([unpart(res_a.results[b_ * 4 + i]["kT_o"])
                               for i in range(4)], axis=1) for b_ in range(B)]
    v_full = [np.concatenate([unpart(res_a.results[b_ * 4 + i]["v_o"])
                              for i in range(4)], axis=0) for b_ in range(B)]

    bo_row = bo.astype(np.float16)[None, :]
    wo16 = part8(Wo.astype(np.float16))

    in_maps_b = []
    for c in range(NC):
        b_, qq = c // 4, c % 4
        in_maps_b.append(_pack_b(qT_full[b_], kT_full[b_], v_full[b_],
                                 wo16, bo_row, qq))

    res_b = run_bass_kernel_spmd(_cache["b"], in_maps_b, core_ids=list(range(NC)))

    y = np.zeros((B, T, D), np.float32)
    for c in range(NC):
        b_, qq = c // 4, c % 4
        blocks = _unpack_y(res_b.results[c]["y"], bo, qq)
        for ib, rows in blocks.items():
            y[b_, ib * 128:(ib + 1) * 128] = rows
    return y
